# revision 1
# baseline (speedup 1.0000x reference)
"""DeepSeekV3-style GQA attention (B=4, S=2048, D=384, H=6, KVH=2, HD=64)
as a Bass/Tile kernel on 8 Trainium2 NeuronCores.

Sharding: (batch, seq-half) -> 8 disjoint shards, no collectives.
Core c handles batch b=c//2 and query rows [qs, qs+1024) with qs=(c%2)*1024.
Keys/values use the full 2048-row sequence of the core's batch; key order is
permuted per-core so the core's own query block always sits at rows 0:1024
(softmax is permutation-invariant over keys, and RoPE tables are permuted
identically on the host).

On-chip layout is feature-major ("transposed") throughout:
  hsT [384, 2048] (host-transposed input) -> Q^T/K^T via Wq/Wk (plus
  host-prepared pair-swapped, sign-folded weight variants for RoPE),
  RoPE applied as  rot = P ele-mul cs + Psw ele-mul sn  on the Vector engine,
  scores computed as S^T tiles [128tk, 1024tq], softmax without
  max-subtraction (scores are O(1) bounded for this problem), exp on the
  Scalar engine, denominator via a ones-column appended to V (rides the
  P@V matmul for free), normalization folded in after P@V.
All matmuls run as float32r (full fp32 data, 1 cycle/row PE streaming).
"""

import os
import sys

import numpy as np

if "/opt/trn_rl_repo" not in sys.path:
    sys.path.insert(0, "/opt/trn_rl_repo")

B, S, D = 4, 2048, 384
H, KVH, HD = 6, 2, 64
GROUPS = H // KVH
N_CORES = 8
SQ = S // 2  # query rows per core (1024)
NT_K = S // 128  # 16 key tiles
ROPE_THETA = 100000.0

_CACHE: dict = {}


def _pair_swap_neg(w: np.ndarray) -> np.ndarray:
    """Columns of w are (head, dim) features; build the RoPE partner matrix:
    col 2i <- -col (2i+1), col (2i+1) <- +col 2i  (within each head)."""
    d, n = w.shape
    wr = w.reshape(d, n // 2, 2)
    return np.stack([-wr[..., 1], wr[..., 0]], axis=-1).reshape(d, n)


def _build_module(do_compile=True):
    import concourse.bass as bass
    import concourse.tile as tile
    from concourse import mybir
    from concourse.bacc import Bacc

    f32 = mybir.dt.float32
    f32r = mybir.dt.float32r

    # Bacc (not plain Bass): its compile() runs generate_event_semaphores,
    # which splits multi-waits down to the 1-wait-per-instruction limit of
    # the TRN2 ISA encodings (walrus rejects >1).
    nc = Bacc()

    hsT = nc.declare_dram_parameter("hsT", [D, S], f32, isOutput=False)
    wq2 = nc.declare_dram_parameter("wq2", [D, 2 * H * HD], f32, isOutput=False)
    wk2 = nc.declare_dram_parameter("wk2", [D, 2 * KVH * HD], f32, isOutput=False)
    wv = nc.declare_dram_parameter("wv", [D, KVH * HD], f32, isOutput=False)
    wo = nc.declare_dram_parameter("wo", [H * HD, D], f32, isOutput=False)
    csK = nc.declare_dram_parameter("csK", [128, S], f32, isOutput=False)
    snK = nc.declare_dram_parameter("snK", [128, S], f32, isOutput=False)
    eye = nc.declare_dram_parameter("eye", [128, 128], f32, isOutput=False)
    out = nc.declare_dram_parameter("out", [SQ, D], f32, isOutput=True)

    KC = D // 128  # 3 contraction chunks of the model dim

    with tile.TileContext(nc) as tc:
        with (
            tc.tile_pool(name="big", bufs=1) as big,
            tc.tile_pool(name="wts", bufs=1) as wts,
            tc.tile_pool(name="work", bufs=4) as work,
            # PSUM budget is 8 banks total, statically split: two pools of
            # 2 slots x [128, 1024]f32 (2 banks) each. Every psum tile below
            # comes from pool pA (tag "a") or pB (tag "b").
            tc.tile_pool(name="pA", bufs=2, space="PSUM") as pA,
            tc.tile_pool(name="pB", bufs=2, space="PSUM") as pB,
            tc.tile_pool(name="pp", bufs=3) as pp,
        ):
            # Every input gets its own DMA into its final (never-reused)
            # tile region, so each DMACopy carries at most one sync wait
            # (the DIRECT2D encoding supports only one). Matmul-consumed
            # tiles then get an in-place DVE copy: walrus requires f32r
            # matmul operands to be produced by an op that "rounds to
            # f32r", and it collapses the matmuls' DMA dependencies onto a
            # single engine semaphore.
            def load_f32r(dst_tile, src_ap):
                nc.sync.dma_start(out=dst_tile, in_=src_ap.bitcast(f32r))
                nc.scalar.copy(dst_tile, dst_tile)

            hsT_sb = big.tile([128, KC, S], f32r)
            wq2_sb = wts.tile([128, KC, 2 * H * HD], f32r)
            wk2_sb = wts.tile([128, KC, 2 * KVH * HD], f32r)
            wv_sb = wts.tile([128, KC, KVH * HD], f32r)
            wo_sb = wts.tile([128, KC, D], f32r)
            eye_sb = wts.tile([128, 128], f32r)
            csK_sb = big.tile([128, S], f32)
            snK_sb = big.tile([128, S], f32)
            k_rot = big.tile([128, S], f32r)
            q_rot = big.tile([128, KC, SQ], f32r)
            v1 = big.tile([128, NT_K, KVH, 2 * HD], f32r)
            oT = big.tile([128, KC, SQ], f32r)  # normalized O^T (2 heads/chunk)

            def mm(out_ap, lhsT_ap, rhs_ap, **kw):
                nc.tensor.matmul(out_ap, lhsT_ap, rhs_ap, **kw)

            def rope(dst_ap, ps_main, ps_swap, cols):
                t1 = work.tile([128, SQ], f32, tag="rope")
                nc.vector.tensor_mul(t1[:, :], ps_main[:, :], csK_sb[:, cols])
                t2 = work.tile([128, SQ], f32, tag="rope")
                nc.vector.tensor_mul(t2[:, :], ps_swap[:, :], snK_sb[:, cols])
                nc.vector.tensor_add(dst_ap, t1[:, :], t2[:, :])

            def emit_K(half):
                # K^T + RoPE for key columns [half*SQ, (half+1)*SQ)
                ps_k = pA.tile([128, SQ], f32, tag="a")
                ps_ksw = pB.tile([128, SQ], f32, tag="b")
                for n in range(SQ // 512):
                    ns = bass.ts(n, 512)
                    gs = bass.ds(half * SQ + n * 512, 512)
                    for c in range(KC):
                        mm(ps_k[:, ns], wk2_sb[:, c, 0:128], hsT_sb[:, c, gs],
                           start=(c == 0), stop=(c == KC - 1))
                    for c in range(KC):
                        mm(ps_ksw[:, ns], wk2_sb[:, c, 128:256], hsT_sb[:, c, gs],
                           start=(c == 0), stop=(c == KC - 1))
                cols = bass.ds(half * SQ, SQ)
                rope(k_rot[:, cols], ps_k, ps_ksw, cols)

            def emit_Q(m):
                # Q^T + RoPE, feature chunk m (heads m and m+3).
                # RoPE scale 1/8 is folded into the exp scale later.
                ps_q = pA.tile([128, SQ], f32, tag="a")
                ps_qsw = pB.tile([128, SQ], f32, tag="b")
                ms = bass.ds(m * 128, 128)
                msw = bass.ds(H * HD + m * 128, 128)
                for n in range(SQ // 512):
                    ns = bass.ts(n, 512)
                    for c in range(KC):
                        mm(ps_q[:, ns], wq2_sb[:, c, ms], hsT_sb[:, c, ns],
                           start=(c == 0), stop=(c == KC - 1))
                    for c in range(KC):
                        mm(ps_qsw[:, ns], wq2_sb[:, c, msw], hsT_sb[:, c, ns],
                           start=(c == 0), stop=(c == KC - 1))
                rope(q_rot[:, m, :], ps_q, ps_qsw, bass.ds(0, SQ))

            vT_sb = None

            def emit_V_proj(half):
                # V^T for key columns of `half`; v1[:, t, g, :] = [V_g | ones].
                # The 64 replicated ones columns make P@V' emit the softmax
                # denominator pre-replicated across 64 partitions, so
                # normalization needs no cross-partition broadcast.
                nonlocal vT_sb
                if vT_sb is None:
                    vT_sb = work.tile([128, S], f32r, tag="vt")
                    nc.vector.memset(v1[:, :, :, HD:2 * HD].bitcast(f32), 1.0)
                    nc.vector.tensor_copy(v1[:, :, :, HD:2 * HD],
                                          v1[:, :, :, HD:2 * HD])
                ps_vt = pA.tile([128, SQ], f32, tag="a")
                for n in range(SQ // 512):
                    ns = bass.ts(n, 512)
                    gs = bass.ds(half * SQ + n * 512, 512)
                    for c in range(KC):
                        mm(ps_vt[0:KVH * HD, ns], wv_sb[:, c, :],
                           hsT_sb[:, c, gs],
                           start=(c == 0), stop=(c == KC - 1))
                cp = nc.scalar.copy if half == 0 else nc.vector.tensor_copy
                cp(
                    vT_sb[0:KVH * HD, bass.ds(half * SQ, SQ)],
                    ps_vt[0:KVH * HD, :],
                )

            def emit_V_tiles(trange):
                for t in trange:
                    ps_v = pB.tile([128, KVH * HD], f32r, tag="b")
                    nc.tensor.matmul(
                        ps_v[:, :],
                        vT_sb[0:KVH * HD, bass.ts(t, 128)],
                        eye_sb[0:KVH * HD, 0:KVH * HD],
                        is_transpose=True,
                    )
                    nc.vector.tensor_copy(
                        v1[:, t, :, 0:HD],
                        ps_v[:, :].rearrange("p (g d) -> p g d", g=KVH),
                    )

            def emit_head(h):
                g = h // GROUPS
                ps_o = pB.tile([2 * HD, SQ], f32, tag="b")
                # scores for tile t are emitted before PV of tile t-1 so
                # the PE runs S(t+1) ahead of PV(t): exp never waits on a
                # freshly-issued scores matmul.
                pending_pv = None
                rhs_q = q_rot[g * HD:(g + 1) * HD, h % 3, :]
                for t in range(NT_K):
                    ps_s = pA.tile([128, SQ], f32, tag="a")
                    lhs_k = k_rot[g * HD:(g + 1) * HD, bass.ts(t, 128)]
                    for n in range(SQ // 512):
                        ns = bass.ts(n, 512)
                        mm(ps_s[:, ns], lhs_k, rhs_q[:, ns])
                    p_sb = pp.tile([128, SQ], f32r)
                    nc.scalar.activation(
                        out=p_sb[:, :], in_=ps_s[:, :],
                        func=mybir.ActivationFunctionType.Exp, scale=0.125,
                    )
                    if pending_pv is not None:
                        pv_t, pv_p = pending_pv
                        for n in range(SQ // 512):
                            ns = bass.ts(n, 512)
                            mm(ps_o[:, ns], v1[:, pv_t, g, :], pv_p[:, ns],
                               start=(pv_t == 0), stop=False)
                    pending_pv = (t, p_sb)
                pv_t, pv_p = pending_pv
                for n in range(SQ // 512):
                    ns = bass.ts(n, 512)
                    mm(ps_o[:, ns], v1[:, pv_t, g, :], pv_p[:, ns],
                       start=False, stop=True)
                # normalize: oT = O^T * (1/denom); denom sits pre-replicated
                # in ps_o rows 64:128 thanks to the ones columns of v1.
                brd = work.tile([HD, SQ], f32, tag="brd")
                nc.vector.reciprocal(brd[:, :], ps_o[HD:2 * HD, :])
                nc.vector.tensor_mul(
                    oT[g * HD:(g + 1) * HD, h % 3, :],
                    ps_o[0:HD, :], brd[:, :],
                )

            # ---- emission order: get head 0's dependencies (K half 0,
            # V half 0, Q chunk 0) done first so the ACT-bound main loop
            # starts early; the rest of the prologue overlaps it.
            for c in range(KC):
                load_f32r(wk2_sb[:, c, :], wk2[bass.ts(c, 128), :])
                load_f32r(hsT_sb[:, c, 0:SQ], hsT[bass.ts(c, 128), 0:SQ])
            nc.sync.dma_start(out=csK_sb[:, :], in_=csK[:, :])
            nc.sync.dma_start(out=snK_sb[:, :], in_=snK[:, :])
            for c in range(KC):
                load_f32r(wv_sb[:, c, :], wv[bass.ts(c, 128), :])
                load_f32r(wq2_sb[:, c, :], wq2[bass.ts(c, 128), :])
            load_f32r(eye_sb[:, :], eye[0:128, :])
            emit_K(0)
            emit_V_proj(0)
            emit_V_tiles(range(0, NT_K // 2))
            emit_Q(0)
            for c in range(KC):
                load_f32r(hsT_sb[:, c, SQ:S], hsT[bass.ts(c, 128), SQ:S])
            emit_K(1)
            emit_V_proj(1)
            emit_V_tiles(range(NT_K // 2, NT_K))
            emit_Q(1)
            for c in range(KC):
                load_f32r(wo_sb[:, c, :], wo[bass.ts(c, 128), :])
            emit_Q(2)
            for h in (0, 3, 1, 4, 2, 5):
                emit_head(h)

            # ---- o_proj, token-major out -------------------------------
            for t in range(SQ // 128):
                ps_f = pA.tile([128, D], f32, tag="a")
                for c in range(KC):
                    mm(ps_f[:, :], oT[:, c, bass.ts(t, 128)], wo_sb[:, c, :],
                       start=(c == 0), stop=(c == KC - 1))
                o_sb = work.tile([128, D], f32, tag="osb")
                nc.scalar.copy(o_sb[:, :], ps_f[:, :])
                nc.sync.dma_start(out=out[bass.ts(t, 128), :], in_=o_sb[:, :])

    if do_compile:
        nc.compile()
    return nc


def _host_inputs(hidden_states, Wq, Wk, Wv, Wo, freqs_cos, freqs_sin):
    """Build the 8 per-core input maps (all numpy, f32)."""
    hs = np.ascontiguousarray(hidden_states, dtype=np.float32)
    cos = np.asarray(freqs_cos, dtype=np.float32)
    sin = np.asarray(freqs_sin, dtype=np.float32)
    # Reorder q heads as (0,3),(1,4),(2,5): head h -> chunk h%3, partition
    # base (h//3)*64 — aligns each q head with its kv group's partition base.
    head_order = [0, 3, 1, 4, 2, 5]
    qcols = np.concatenate([np.arange(h * HD, (h + 1) * HD) for h in head_order])
    Wq = np.asarray(Wq, dtype=np.float32)
    wq2 = np.concatenate(
        [Wq[:, qcols], _pair_swap_neg(Wq)[:, qcols]], axis=1
    ).astype(np.float32)
    wk2 = np.concatenate([Wk, _pair_swap_neg(np.asarray(Wk))], axis=1).astype(np.float32)
    wv = np.ascontiguousarray(Wv, dtype=np.float32)
    wo = np.ascontiguousarray(np.asarray(Wo, dtype=np.float32)[qcols, :])
    eye = np.eye(128, dtype=np.float32)
    row_sel = (np.arange(128) % 64) // 2  # feature row j -> freq index

    in_maps = []
    for c in range(N_CORES):
        b, half = c // 2, c % 2
        perm = np.r_[half * SQ:(half + 1) * SQ, (1 - half) * SQ:(2 - half) * SQ] % S
        hsT = np.ascontiguousarray(hs[b][perm].T)  # [D, S]
        cosP, sinP = cos[perm], sin[perm]  # [S, 32]
        csK = np.ascontiguousarray(cosP[:, row_sel].T)  # [128, S]
        snK = np.ascontiguousarray(sinP[:, row_sel].T)
        in_maps.append({
            "hsT": hsT, "wq2": wq2, "wk2": wk2, "wv": wv, "wo": wo,
            "csK": csK, "snK": snK, "eye": eye,
        })
    return in_maps


def get_module():
    if "nc" not in _CACHE:
        _CACHE["nc"] = _build_module()
    return _CACHE["nc"]


def run_on_hw(in_maps, **kw):
    from concourse.bass_utils import run_bass_kernel_spmd

    nc = get_module()
    return run_bass_kernel_spmd(nc, in_maps, core_ids=list(range(N_CORES)), **kw)


def kernel(hidden_states, Wq, Wk, Wv, Wo, freqs_cos, freqs_sin):
    in_maps = _host_inputs(hidden_states, Wq, Wk, Wv, Wo, freqs_cos, freqs_sin)
    res = run_on_hw(in_maps)
    out = np.empty((B, S, D), dtype=np.float32)
    for c in range(N_CORES):
        b, half = c // 2, c % 2
        out[b, half * SQ:(half + 1) * SQ, :] = res.results[c]["out"]
    return out

